# revision 9
# baseline (speedup 1.0000x reference)
# Trainium2 Bass kernel for nn_ExpandFrame: gaussian-upsampling attention
#   e = cumsum(duration, -1); c = e - 0.5*round(duration)
#   logits[b,n,t] = temp * (t - c[b,n])^2 ;  temp = -1/(5*sqrt(duration[0,0]))
#   w = softmax(logits, axis=n) ;  out[b,d,t] = sum_n w[b,n,t] * hidden[b,n,d]
#
# Strategy: data-parallel over batch B=16 across 8 cores (2 batches/core).
# v4 design:
#  - softmax weights are emitted directly in [n, t] layout: per (tp, k)
#    window diff = Rs + col on GpSimd/VectorE (Rs[j] = s*(j-512) host-sent,
#    col = s*(anchor_tp - c_n) a host-sent column), pos = diff*diff, then
#    ONE batched exp per (b, tp) on ScalarE over the tp's contiguous pos
#    scratch -> wk fp16 in SBUF.  No PE transposes, no PSUM staging.
#  - contraction: banded partial-width matmuls (stationary hid[k] fp16,
#    moving wk) into per-(dci,tp) [128,1024] f32 PSUM, per-bank start/stop.
#  - dci-outer loop: each dci's full 4096-t row completes after 4 tp
#    sweeps -> 1MB contiguous out-DMA per (b, dci), stream starts early.
#  - PSUM->SBUF evacuation greedily balanced between ScalarE and VectorE.
#  - input DMAs strictly ordered on the sync queue (cols, Rs, hid halves)
#    so the first sweep's dependencies land ASAP; PE pre-warmed with dummy
#    matmuls so HAM reaches full clock before the real stream.
#  - softmax denominators S on host (exact f32 replication); columns whose
#    S underflows (~250 of 65536) are recomputed exactly on host.
import numpy as np

B, N, D, T = 16, 1024, 1024, 4096
NCORES = 8
BPC = B // NCORES        # batches per core
P = 128                  # partitions
KN = N // P              # 8 n-chunks
TC = 128                 # t-chunk granularity for window search
NTC = T // TC            # 32
TP = 1024                # t-span per (dci, tp) PSUM accumulation
NTP = T // TP            # 4
TT = 512                 # PSUM bank width (f32)
CUT = 9.0                # exp cutoff for window margin
SMIN = 1e-4              # host fixup threshold on softmax denominator
N_STT_DVE = 8            # first N wide spans' diff/pos run on VectorE
EV_ACT, EV_DVE = 1.10, 1.24   # evac cost model (us) for greedy balancing
EXP_US = 1.41            # batched exp cost per tp


def _host_prep(duration):
    """Centers, temp, static band windows (shared across all batches), and
    the per-span col tensor for the on-device softmax."""
    dur = np.asarray(duration, dtype=np.float32)
    e = np.cumsum(dur, axis=-1, dtype=np.float32)
    c = (e - np.float32(0.5) * np.round(dur)).astype(np.float32)   # [B, N]
    d00 = float(dur[0, 0])
    temp = -1.0 / (5.0 * np.sqrt(d00))
    s = float(np.sqrt(-temp))
    margin = int(np.ceil(np.sqrt(CUT) / s)) + 2

    lo = np.empty((B, NTC), dtype=np.int64)
    hi = np.empty((B, NTC), dtype=np.int64)
    for b in range(B):
        t0s = np.arange(NTC) * TC
        lo[b] = np.searchsorted(c[b], t0s - margin, side="left")
        hi[b] = np.searchsorted(c[b], t0s + (TC - 1) + margin, side="right")
    ulo = np.minimum(lo.min(axis=0), N - 1)
    uhi = np.maximum(hi.max(axis=0), ulo + 1)
    klo = ulo // P
    khi = (uhi + P - 1) // P

    # per-tp spans: for each 1024-t window, per chunk k the contiguous range
    # of covered t-chunks (tp-local cols), plus 512-bank matmul pieces
    spans = []   # spans[tp] = list of (k, c0, c1) with c in [0, 1024)
    mms = []     # mms[tp][i] = (k, [(tt, g0, g1)...]) with g in [0, 512)
    for tp in range(NTP):
        tcs = list(range(tp * 8, (tp + 1) * 8))
        Klo = min(klo[t] for t in tcs)
        Khi = max(khi[t] for t in tcs)
        sp = []
        for k in range(Klo, Khi):
            cov = [t - tp * 8 for t in tcs if klo[t] <= k < khi[t]]
            sp.append((int(k), min(cov) * TC, (max(cov) + 1) * TC))
        spans.append(sp)
        per_k = []
        for (k, c0, c1) in sp:
            pieces = []
            for tt in range(2):
                lo_t, hi_t = tt * TT, (tt + 1) * TT
                g0, g1 = max(c0, lo_t), min(c1, hi_t)
                if g0 < g1:
                    pieces.append((tt, g0 - lo_t, g1 - lo_t))
            per_k.append((k, pieces))
        mms.append(per_k)

    # flat span list, per-tp scratch offsets, and the col tensor
    flat = [(tp, k, c0, c1) for tp in range(NTP) for (k, c0, c1) in spans[tp]]
    nsp = len(flat)
    offs = []     # offs[tp] = {k: offset into the tp's contiguous scratch}
    widths = []   # widths[tp] = total scratch width
    for tp in range(NTP):
        o, cur = {}, 0
        for (k, c0, c1) in spans[tp]:
            o[k] = cur
            cur += c1 - c0
        offs.append(o)
        widths.append(cur)
    sf = np.float32(s)
    cols = np.empty((B, P, nsp), dtype=np.float32)   # s * (anchor - c_n)
    for si, (tp, k, c0, c1) in enumerate(flat):
        anchor = np.float32(tp * TP + TP // 2)
        cols[:, :, si] = (sf * (anchor - c[:, k * P:(k + 1) * P])
                          ).astype(np.float32)
    return c, s, spans, mms, flat, offs, widths, cols


def _build(nc, s, spans, mms, flat, offs, widths, nsp):
    import concourse.tile as tile
    import concourse.mybir as mybir

    f32 = mybir.dt.float32
    f16 = mybir.dt.float16
    AF = mybir.ActivationFunctionType
    ALU = mybir.AluOpType

    hidd = nc.dram_tensor("hidden", [BPC, P, KN, D], f16,
                          kind="ExternalInput").ap()
    colsd = nc.dram_tensor("cols", [BPC, P, nsp], f32,
                           kind="ExternalInput").ap()
    rsd = nc.dram_tensor("rs", [P, TP], f32, kind="ExternalInput").ap()
    outd = nc.dram_tensor("out", [BPC, D, T], f16, kind="ExternalOutput").ap()

    sidx = {}  # (tp, k) -> flat span index
    for si, (tp, k, c0, c1) in enumerate(flat):
        sidx[(tp, k)] = si

    with tile.TileContext(nc) as tc:
        import contextlib
        with contextlib.ExitStack() as ctx:
            constp = ctx.enter_context(tc.tile_pool(name="const", bufs=1))
            hidp = ctx.enter_context(tc.tile_pool(name="hid", bufs=2))
            colp = ctx.enter_context(tc.tile_pool(name="colp", bufs=2))
            dfp = ctx.enter_context(tc.tile_pool(name="dfp", bufs=4))
            qp = ctx.enter_context(tc.tile_pool(name="qp", bufs=3))
            wkp = ctx.enter_context(tc.tile_pool(name="wk", bufs=6))
            osbp = ctx.enter_context(tc.tile_pool(name="osb", bufs=3))
            pop = ctx.enter_context(tc.tile_pool(name="po", bufs=4,
                                                 space="PSUM"))

            # input DMAs in dependency-priority order on the sync HWDGE
            rs = constp.tile([P, TP], f32)
            col_sb = [None, None]
            hid_sb = [None, None]
            ct0 = colp.tile([P, nsp], f32, tag="c", name="ct")
            nc.sync.dma_start(ct0[:], colsd[0])
            col_sb[0] = ct0
            nc.sync.dma_start(rs[:], rsd)
            ht0 = hidp.tile([P, KN, D], f16, tag="hid", name="ht")
            nc.sync.dma_start(ht0[:, 0:4, :], hidd[0, :, 0:4, :])
            nc.sync.dma_start(ht0[:, 4:8, :], hidd[0, :, 4:8, :])
            hid_sb[0] = ht0
            ct1 = colp.tile([P, nsp], f32, tag="c", name="ct")
            nc.sync.dma_start(ct1[:], colsd[1])
            col_sb[1] = ct1
            ht1 = hidp.tile([P, KN, D], f16, tag="hid", name="ht")
            nc.sync.dma_start(ht1[:, 0:4, :], hidd[1, :, 0:4, :])
            nc.sync.dma_start(ht1[:, 4:8, :], hidd[1, :, 4:8, :])
            hid_sb[1] = ht1

            # warm the ACT exp spline table, and the PE HAM clock with
            # dummy matmuls on a zero tile so the real stream starts at
            # full rate
            warm = constp.tile([P, 1], f32)
            nc.scalar.activation(warm[:], warm[:], AF.Exp, bias=0.0,
                                 scale=-1.0)
            wz = constp.tile([P, P], f16)
            nc.gpsimd.memset(wz[:], 0.0)
            powarm = pop.tile([P, 2, TT], f32, tag="po")
            for i in range(16):
                nc.tensor.matmul(powarm[:, 0, 0:P], wz[:], wz[:],
                                 start=True, stop=True)

            wks = [dict(), dict()]   # b -> {tp: (wk tile, {k: off})}
            eng_load = [0.0, 0.0]    # running (ACT, DVE) engine load
            stt_i = [0]

            def softmax_tp(b, tp):
                # diff = Rs + col, pos = diff^2 per span into one contiguous
                # scratch; single batched exp per (b, tp) -> wk fp16
                w_tot = widths[tp]
                q = qp.tile([P, w_tot], f32, tag="q")
                for (k, c0, c1) in spans[tp]:
                    si = sidx[(tp, k)]
                    w = c1 - c0
                    o = offs[tp][k]
                    col = col_sb[b][:, si:si + 1]
                    wide = w > 256
                    if wide and stt_i[0] < N_STT_DVE:
                        eng = nc.vector
                    else:
                        eng = nc.gpsimd
                    if wide:
                        stt_i[0] += 1
                    df = dfp.tile([P, w], f32, tag="df")
                    eng.tensor_scalar(df[:], rs[:, c0:c1], 1.0, col,
                                      op0=ALU.mult, op1=ALU.add)
                    eng.tensor_tensor(q[:, o:o + w], df[:], df[:],
                                      op=ALU.mult)
                wk = wkp.tile([P, w_tot], f16, tag="wk", name="wk")
                nc.scalar.activation(wk[:], q[:], AF.Exp, bias=0.0,
                                     scale=-1.0)
                eng_load[0] += EXP_US
                wks[b][tp] = wk

            def sweep_tp(b, dci, tp, dst, last=False):
                po = pop.tile([P, 2, TT], f32, tag="po")
                per_k = mms[tp]
                span_c0 = {k: c0 for (k, c0, c1) in spans[tp]}
                n_bank = [sum(1 for _, ps in per_k for t, _, _ in ps
                              if t == tt) for tt in range(2)]
                seen = [0, 0]
                wk = wks[b][tp]
                for k, pieces in per_k:
                    o = offs[tp][k] - span_c0[k]
                    for tt, g0, g1 in pieces:
                        seen[tt] += 1
                        nc.tensor.matmul(
                            po[:, tt, g0:g1],
                            hid_sb[b][:, k, dci * P:(dci + 1) * P],
                            wk[:, o + g0 + tt * TT:o + g1 + tt * TT],
                            start=(seen[tt] == 1),
                            stop=(seen[tt] == n_bank[tt]),
                            skip_group_check=True)
                # greedy engine balance for the PSUM->SBUF evacuation
                if eng_load[0] + EV_ACT <= eng_load[1] + EV_DVE:
                    eng_load[0] += EV_ACT
                    nc.scalar.copy(dst, po[:])
                else:
                    eng_load[1] += EV_DVE
                    nc.vector.tensor_copy(dst, po[:])

            softmax_tp(0, 0)
            softmax_tp(0, 1)
            softmax_tp(0, 2)
            softmax_tp(0, 3)
            ahead = {(0, 1): (1, 0), (0, 3): (1, 1),
                     (0, 5): (1, 2), (0, 7): (1, 3)}
            for b in range(BPC):
                for dci in range(KN):
                    if (b, dci) in ahead:
                        softmax_tp(*ahead[(b, dci)])
                    osb = osbp.tile([P, NTP, TP], f16, tag="osb", name="osb")
                    final = (b == BPC - 1 and dci == KN - 1)
                    for tp in range(NTP):
                        sweep_tp(b, dci, tp, osb[:, tp, :])
                        if final:
                            # drain the very last row per-tp so the tail DMA
                            # is short
                            nc.sync.dma_start(
                                outd[b, dci * P:(dci + 1) * P,
                                     tp * TP:(tp + 1) * TP],
                                osb[:, tp, :])
                    if not final:
                        nc.sync.dma_start(outd[b, dci * P:(dci + 1) * P, :],
                                          osb[:])
    return nc


def _host_s(spans, flat, offs, widths, cols, rs_host):
    """Softmax denominators, replicating the device's f32 window math and
    the fp16 rounding of the stored weights."""
    S = np.zeros((B, T), dtype=np.float32)
    for si, (tp, k, c0, c1) in enumerate(flat):
        rsw = rs_host[c0:c1][None, :]                        # [1, w]
        col = cols[:, :, si][:, :, None]                     # [B, 128, 1]
        df = (rsw[:, None, :] + col).astype(np.float32)      # [B, 128, w]
        pos = (df * df).astype(np.float32)
        e16 = np.exp(-pos.astype(np.float64)).astype(np.float16)
        S[:, tp * TP + c0:tp * TP + c1] += e16.astype(np.float32).sum(axis=1)
    return S


def _host_fix(out, S, c, hidden, temp):
    """Exact recompute of columns whose windowed denominator underflowed."""
    bad_b, bad_t = np.where(S < SMIN)
    if len(bad_b) == 0:
        return
    for b in np.unique(bad_b):
        ts = bad_t[bad_b == b]
        lg = temp * (ts[:, None].astype(np.float64)
                     - c[b][None, :].astype(np.float64)) ** 2    # [nt, N]
        lg -= lg.max(axis=1, keepdims=True)
        w = np.exp(lg)
        w /= w.sum(axis=1, keepdims=True)
        out[b][:, ts] = (hidden[b].astype(np.float64).T @ w.T).astype(
            np.float32)


def _run(inputs, trace=False):
    import concourse.bacc as bacc
    from concourse.bass_utils import run_bass_kernel_spmd

    hidden = np.asarray(inputs["hidden"], dtype=np.float32)
    duration = np.asarray(inputs["duration"], dtype=np.float32)

    c, s, spans, mms, flat, offs, widths, cols = _host_prep(duration)
    nsp = len(flat)
    sf = np.float32(s)
    rs_host = (np.float64(sf) * np.arange(TP, dtype=np.float64)
               + np.float64(np.float32(-512.0 * sf))).astype(np.float32)
    rs_full = np.ascontiguousarray(np.broadcast_to(rs_host, (P, TP)))
    # host pre-shuffle: [b, n, d] -> [b, p, k, d] so the DMA is contiguous
    hid_f16 = hidden.astype(np.float16)
    hid_shuf = np.ascontiguousarray(
        hid_f16.reshape(B, KN, P, D).transpose(0, 2, 1, 3))

    nc = bacc.Bacc("TRN2", target_bir_lowering=False, debug=False,
                   enable_asserts=False, num_devices=NCORES)
    _build(nc, s, spans, mms, flat, offs, widths, nsp)
    nc.compile()

    in_maps = []
    for i in range(NCORES):
        bs = slice(i * BPC, (i + 1) * BPC)
        in_maps.append({
            "hidden": hid_shuf[bs],
            "cols": np.ascontiguousarray(cols[bs]),
            "rs": rs_full,
        })
    res = run_bass_kernel_spmd(nc, in_maps, core_ids=list(range(NCORES)),
                               trace=trace)
    out_f16 = np.concatenate([res.results[i]["out"] for i in range(NCORES)],
                             axis=0)
    S = _host_s(spans, flat, offs, widths, cols, rs_host)
    out = out_f16.astype(np.float32) / np.maximum(S, 1e-30)[:, None, :]
    temp = -1.0 / (5.0 * np.sqrt(np.float64(duration[0, 0])))
    _host_fix(out, S, c, hidden, temp)
    return out, res


def kernel(**inputs) -> np.ndarray:
    out, _ = _run(inputs, trace=False)
    return out
